# revision 3
# baseline (speedup 1.0000x reference)
"""Trainium2 Bass kernel for nn_ConstrainLoss (weighted logsumexp over a
Gaussian-kernel cost matrix, dotted with row weights -> scalar).

Math:
    sq_ij = |x_i - xo_j|^2          (relu clamp in the reference never fires)
    C_ij  = -2*sq_ij + log(w_obs_j)          (inv_two_s2 == 2.0)
          = 4*x_i.xo_j + a_j + b_i
      a_j = -2*|xo_j|^2 + log(w_obs_j)
      b_i = -2*|x_i|^2            (pulls out of the LSE entirely -> host term)
    out   = -sum_i x_w_i * (b_i + logsumexp_j(T_ij)),  T_ij = 4*x_i.xo_j + a_j

Device kernel (per core, rows sharded 2048/core):
    T tile: fp8e4 DoubleRow matmul (2 cols/cycle). Contraction packed into
      [128 partitions, 2 k-subtiles]: subtile 0 = hi/lo compensated splits
      (hi*hi, hi*lo, lo*hi, lo*lo of 4x and xo, 32 rows each); subtile 1 =
      4 fp8 refinement rows of a_j (x-side ones). |T| error ~0.1-0.3 which
      the final weighted sum averages away (validated 2e-4 rel on host sim).
    Columns of each 128-row block are split between two exp pipelines:
      - ACT chunks: ScalarE exp with bias=-shift and fused accum_out
        (1 elem/cycle/lane) straight from PSUM.
      - chain chunks: DVE tensor_scalar computes y16 = A16*T + (B16-A16*s)
        with output dtype uint16: the fp32->uint16 convert saturates
        negatives to 0, which IS the Schraudolph clamp. The uint16 holds the
        bf16 bit pattern of ~exp(T-s) (A16=2^7/ln2, B16 zero-mean
        calibrated). A second DVE tensor_scalar over the bf16-bitcast view
        (16-bit 2x/4x mode) accumulates the block's chain columns into one
        accum col.
    Splitting exp across ACT+DVE is what beats the ACT-only baseline (ACT
    was 92% busy at 268us); fp8 halves PE time so the stalled-PE mid-pstate
    (1.2GHz) still keeps PE ahead of the consumers.
    lse_i = shift_i + ln(S) on host in fp64 (S = sum of accum cols).

Host: result = -(sum_cores sum(lse*w) + sum_i b_i*x_w_i)
"""

import sys

if "/opt/trn_rl_repo" not in sys.path:
    sys.path.insert(0, "/opt/trn_rl_repo")

import re
from contextlib import ExitStack

import ml_dtypes
import numpy as np

import bass_rust
import concourse.bass as bass
import concourse.tile as tile
from concourse import mybir
from concourse.bass_utils import run_bass_kernel_spmd
from concourse.tile import ScopedClock, TileContext


def _patched_drain_and_barrier(self, tick_clock, wait_clock):
    """The walrus build in this container rejects >1 sync wait on one
    instruction ("Too many sync wait commands" on Tile's kernel-tail drain).
    Split the tail-drain waits onto individual SP nops, one wait each."""
    gc = tick_clock.global_clock
    ticks = [int(s) for s in re.findall(r"\d+", repr(gc))]
    for i, t in enumerate(ticks):
        if t > 0:
            nop = self.nc.sync.nop(hint="split_wait", nofuse=True)
            vc = bass_rust.VectorClock()
            vc.require_at_least(i, t)
            wait_clock.add_sem_waits(nop.ins, ScopedClock({None: vc}))
    self.nc.sync.drain()
    self.nc.all_engine_barrier()
    assert self.sems is not None
    popped = self.nc._tile_sem_poison_stack.pop()
    assert popped is self._sem_poison
    self.nc.clear_and_free_semaphores(list(self.sems.allocated().values()))
    self.nc.all_engine_barrier()


TileContext._drain_and_barrier = _patched_drain_and_barrier

_MAX_WAITS = 1  # this walrus build rejects >1 sync wait per instruction


def _split_excess_waits(nc):
    """Move excess sync waits (beyond _MAX_WAITS) from any instruction onto
    freshly inserted same-engine nops placed immediately before it. The
    engine executes the nops (waiting) first, so semantics are unchanged."""
    counter = [0]
    for f in nc.m.functions:
        for blk in f.blocks:
            il = blk.instructions  # live list
            i = 0
            while i < len(il):
                ins = il[i]
                si = ins.sync_info
                if si is not None and len(si.on_wait) > _MAX_WAITS:
                    waits = list(si.on_wait)
                    keep = waits[-_MAX_WAITS:]
                    excess = waits[: -_MAX_WAITS]
                    pos = i
                    for j in range(0, len(excess), _MAX_WAITS):
                        counter[0] += 1
                        nop = mybir.InstNoOp(
                            name=f"I-splitw{counter[0]}", ins=[], outs=[]
                        )
                        nop.engine = ins.engine
                        nop.sync_info = mybir.SyncInfo(
                            on_wait=excess[j : j + _MAX_WAITS], on_update=[]
                        )
                        il.insert(pos, nop)
                        pos += 1
                        i += 1
                    ins.sync_info = mybir.SyncInfo(
                        on_wait=keep, on_update=list(si.on_update)
                    )
                i += 1


N, M, D = 16384, 16384, 32
NCORES = 8
N_LOC = N // NCORES  # 2048 rows per core
BLK = 128  # rows per block (psum partitions)
NBLK = N_LOC // BLK  # 16
CHUNK = 2048  # columns per consumer instruction (4 psum banks)
NCHUNK = M // CHUNK  # 8 per block
MMW = 512  # matmul free width (1 psum bank)
SEED_W = 512  # seed max over first SEED_W columns

LN2 = float(np.log(2.0))
A16 = 2.0**7 / LN2  # Schraudolph scale for bf16 bit patterns
C16 = 7.392  # zero-mean calibration (see numcheck2)
B16 = 127 * 2.0**7 - C16

# Per-block chunk pattern: True = ACT (native exp), False = chain (DVE).
# ~60% ACT / 40% chain balances ACT (0.83ns/elem) vs DVE chain
# (1.04 + ~0.3 ns/elem). 13 blocks 5A/3C + 3 blocks 4A/4C -> 77A/51C.
_PAT_5A = (True, False, True, True, False, True, True, False)
_PAT_4A = (True, False, True, False, True, False, True, False)
_BLOCK_PAT = [_PAT_4A if b % 5 == 4 else _PAT_5A for b in range(NBLK)]
SLOTS = 8  # accum columns reserved per block in s_out

F32 = mybir.dt.float32
U16 = mybir.dt.uint16
BF16 = mybir.dt.bfloat16
FP8 = mybir.dt.float8e4
Alu = mybir.AluOpType

_cache = {}


def _build_bass():
    nc = bass.Bass()
    xT_d = nc.declare_dram_parameter("xT", [128, 2, N_LOC], FP8, isOutput=False)
    xoT_d = nc.declare_dram_parameter("xoT", [128, 2, M], FP8, isOutput=False)
    negsh_d = nc.declare_dram_parameter("negsh", [BLK, NBLK], F32, isOutput=False)
    bvec_d = nc.declare_dram_parameter("bvec", [BLK, NBLK], F32, isOutput=False)
    s_d = nc.declare_dram_parameter("s_out", [BLK, NBLK * SLOTS], F32, isOutput=True)

    with tile.TileContext(nc) as tc, ExitStack() as ctx:
        singles = ctx.enter_context(tc.tile_pool(name="singles", bufs=1))
        zpool = ctx.enter_context(tc.tile_pool(name="zp", bufs=2))
        psp = ctx.enter_context(tc.tile_pool(name="ps", bufs=2, space="PSUM"))

        xo_sb = singles.tile([128, 2, M], FP8)
        x_sb = singles.tile([128, 2, N_LOC], FP8)
        s_full = singles.tile([BLK, NBLK * SLOTS], F32)
        negsh_full = singles.tile([BLK, NBLK], F32)
        bvec_full = singles.tile([BLK, NBLK], F32)
        junk = singles.tile([128, 4 * CHUNK], BF16)

        # Spread input DMAs across engine queues so they land in parallel;
        # the first matmuls depend only on x + vectors + xo piece 0.
        nc.sync.dma_start(out=negsh_full, in_=negsh_d[:, :])
        nc.sync.dma_start(out=bvec_full, in_=bvec_d[:, :])
        nc.sync.dma_start(out=x_sb, in_=xT_d[:, :, :])
        NPIECE = 8
        PW = M // NPIECE
        dma_engines = [nc.sync, nc.gpsimd]
        for p in range(NPIECE):
            dma_engines[p % len(dma_engines)].dma_start(
                out=xo_sb[:, :, p * PW : (p + 1) * PW],
                in_=xoT_d[:, :, p * PW : (p + 1) * PW],
            )

        for b in range(NBLK):
            pat = _BLOCK_PAT[b]
            nchain = sum(1 for t in pat if not t)
            negsh = negsh_full[:, b : b + 1]
            bvec = bvec_full[:, b : b + 1]
            s_all = s_full[:, b * SLOTS : (b + 1) * SLOTS]
            lhsT = x_sb[:, :, b * BLK : (b + 1) * BLK]
            zt = zpool.tile([128, nchain * CHUNK], U16, tag="z")
            zi = 0
            slot = 0
            for ci, is_act in enumerate(pat):
                ps = psp.tile([BLK, CHUNK], F32, tag="ps")
                for c in range(CHUNK // MMW):
                    j0 = b * 0 + ci * CHUNK + c * MMW
                    nc.tensor.matmul(
                        out=ps[:, c * MMW : (c + 1) * MMW],
                        lhsT=lhsT,
                        rhs=xo_sb[:, :, j0 : j0 + MMW],
                        start=True,
                        stop=True,
                        perf_mode=mybir.MatmulPerfMode.DoubleRow,
                    )
                if is_act:
                    nc.scalar.activation(
                        out=ps,
                        in_=ps,
                        func=mybir.ActivationFunctionType.Exp,
                        bias=negsh,
                        scale=1.0,
                        accum_out=s_all[:, slot : slot + 1],
                    )
                    slot += 1
                else:
                    nc.vector.tensor_scalar(
                        out=zt[:, zi * CHUNK : (zi + 1) * CHUNK],
                        in0=ps,
                        scalar1=float(A16),
                        scalar2=bvec,
                        op0=Alu.mult,
                        op1=Alu.add,
                    )
                    zi += 1
            # one accumulate pass over the block's chain columns (bf16 view)
            nc.vector.tensor_scalar(
                out=junk[:, : nchain * CHUNK],
                in0=zt.bitcast(BF16),
                scalar1=1.0,
                scalar2=0.0,
                op0=Alu.mult,
                op1=Alu.add,
                accum_out=s_all[:, slot : slot + 1],
            )
            nc.sync.dma_start(
                out=s_d[:, b * SLOTS : (b + 1) * SLOTS],
                in_=s_all,
            )

    _split_excess_waits(nc)
    return nc


def _get_nc():
    if "nc" not in _cache:
        _cache["nc"] = _build_bass()
    return _cache["nc"]


def _fp8(v):
    return v.astype(ml_dtypes.float8_e4m3)


def _prep_inputs(x, x_w, x_obs, x_obs_w):
    x = np.ascontiguousarray(x, dtype=np.float32)
    x_obs = np.ascontiguousarray(x_obs, dtype=np.float32)
    x_obs_w = np.ascontiguousarray(x_obs_w, dtype=np.float32)

    c = np.sum(x_obs * x_obs, axis=1, dtype=np.float32)
    a = (-2.0 * c + np.log(x_obs_w)).astype(np.float32)

    # fp8 compensated splits
    x4 = 4.0 * x
    xh = _fp8(x4)
    xl = _fp8(x4 - xh.astype(np.float32))
    xoh = _fp8(x_obs)
    xol = _fp8(x_obs - xoh.astype(np.float32))
    a1 = _fp8(a)
    a2 = _fp8(a - a1.astype(np.float32))
    a3 = _fp8(a - a1.astype(np.float32) - a2.astype(np.float32))
    a4 = _fp8(
        a - a1.astype(np.float32) - a2.astype(np.float32) - a3.astype(np.float32)
    )

    # xoT[p, s, j]: s0 = [hi,lo,hi,lo][p//32](xo)[j, p%32]; s1 p<4 = a_{p+1}
    xoT = np.zeros((128, 2, M), dtype=ml_dtypes.float8_e4m3)
    xoT[0:32, 0, :] = xoh.T
    xoT[32:64, 0, :] = xol.T
    xoT[64:96, 0, :] = xoh.T
    xoT[96:128, 0, :] = xol.T
    xoT[0, 1, :] = a1
    xoT[1, 1, :] = a2
    xoT[2, 1, :] = a3
    xoT[3, 1, :] = a4

    # Host-side LSE shift: exact max of T over the first SEED_W columns.
    # On this data max_j T - shift <= ~70, well under both the fp32 exp
    # range (ACT path) and the uint16 bitcast ceiling (~89.5).
    T_seed = (4.0 * (x @ x_obs[:SEED_W].T) + a[None, :SEED_W]).astype(np.float32)
    shift = T_seed.max(axis=1)  # [N]

    one = np.ones((), dtype=ml_dtypes.float8_e4m3)
    in_maps = []
    for core in range(NCORES):
        sl = slice(core * N_LOC, (core + 1) * N_LOC)
        # xT[p, s, m]: s0 = [hi,hi,lo,lo][p//32](4x)[m, p%32]; s1 p<4 = 1
        xT = np.zeros((128, 2, N_LOC), dtype=ml_dtypes.float8_e4m3)
        xT[0:32, 0, :] = xh[sl].T
        xT[32:64, 0, :] = xh[sl].T
        xT[64:96, 0, :] = xl[sl].T
        xT[96:128, 0, :] = xl[sl].T
        xT[0:4, 1, :] = one
        sh = shift[sl].reshape(NBLK, BLK).T  # [128, NBLK]
        negsh = np.ascontiguousarray(-sh, dtype=np.float32)
        bvec = np.ascontiguousarray(
            (np.float32(B16) - np.float32(A16) * sh), dtype=np.float32
        )
        in_maps.append({"xT": xT, "xoT": xoT, "negsh": negsh, "bvec": bvec})
    return in_maps, shift


def kernel(x, x_w, x_obs, x_obs_w, _trace=False, _tmpdir=None):
    nc = _get_nc()
    in_maps, shift = _prep_inputs(x, x_w, x_obs, x_obs_w)
    res = run_bass_kernel_spmd(
        nc,
        in_maps,
        core_ids=list(range(NCORES)),
        trace=_trace,
        tmpdir=_tmpdir,
    )
    _cache["last_results"] = res
    # host epilogue (fp64): lse_i = shift_i + log(S_i) + b_i
    x = np.ascontiguousarray(x, dtype=np.float32)
    x_w64 = np.ascontiguousarray(x_w, dtype=np.float32).astype(np.float64)
    r = np.sum(x.astype(np.float64) * x, axis=1)
    total = float(np.dot(-2.0 * r, x_w64))
    nvalid = [sum(1 for t in pat if t) + 1 for pat in _BLOCK_PAT]
    for core in range(NCORES):
        out = res.results[core]
        s = out["s_out"].astype(np.float64).reshape(BLK, NBLK, SLOTS)
        S = np.zeros((BLK, NBLK), np.float64)
        for b in range(NBLK):
            S[:, b] = s[:, b, : nvalid[b]].sum(axis=1)
        sl = slice(core * N_LOC, (core + 1) * N_LOC)
        sh = shift[sl].astype(np.float64).reshape(NBLK, BLK).T
        lse = sh + np.log(S)
        w_arr = x_w64[sl].reshape(NBLK, BLK).T
        total += float((lse * w_arr).sum())
    return np.asarray(-total, dtype=np.float32)


# revision 9
# speedup vs baseline: 1.4071x; 1.4071x over previous
"""Trainium2 Bass kernel for nn_ConstrainLoss (weighted logsumexp over a
Gaussian-kernel cost matrix, dotted with row weights -> scalar).

Math:
    sq_ij = |x_i - xo_j|^2          (relu clamp in the reference never fires)
    C_ij  = -2*sq_ij + log(w_obs_j)          (inv_two_s2 == 2.0)
          = 4*x_i.xo_j + a_j + b_i
      a_j = -2*|xo_j|^2 + log(w_obs_j)
      b_i = -2*|x_i|^2            (pulls out of the LSE entirely -> host term)
    out   = -sum_i x_w_i * (b_i + logsumexp_j(T_ij)),  T_ij = 4*x_i.xo_j + a_j

Device kernel (per core, rows sharded 2048/core):
    T tile: fp8e4 DoubleRow matmul (2 cols/cycle). Contraction packed into
      [128 partitions, 2 k-subtiles]: subtile 0 = hi/lo compensated splits
      (hi*hi, hi*lo, lo*hi, lo*lo of 4x and xo, 32 rows each); subtile 1 =
      4 fp8 refinement rows of a_j (x-side ones). |T| error ~0.1-0.3 which
      the final weighted sum averages away (validated 2e-4 rel on host sim).
    Columns of each 128-row block are split between two exp pipelines:
      - ACT chunks: ScalarE exp with bias=-shift and fused accum_out
        (1 elem/cycle/lane) straight from PSUM.
      - chain chunks: DVE tensor_scalar computes y16 = A16*T + (B16-A16*s)
        with output dtype uint16: the fp32->uint16 convert saturates
        negatives to 0, which IS the Schraudolph clamp. The uint16 holds the
        bf16 bit pattern of ~exp(T-s) (A16=2^7/ln2, B16 zero-mean
        calibrated). A second DVE tensor_scalar over the bf16-bitcast view
        (16-bit 2x/4x mode) accumulates the block's chain columns into one
        accum col.
    Splitting exp across ACT+DVE is what beats the ACT-only baseline (ACT
    was 92% busy at 268us); fp8 halves PE time so the stalled-PE mid-pstate
    (1.2GHz) still keeps PE ahead of the consumers.
    lse_i = shift_i + ln(S) on host in fp64 (S = sum of accum cols).

Host: result = -(sum_cores sum(lse*w) + sum_i b_i*x_w_i)
"""

import sys

if "/opt/trn_rl_repo" not in sys.path:
    sys.path.insert(0, "/opt/trn_rl_repo")

import re
from contextlib import ExitStack

import ml_dtypes
import numpy as np

import bass_rust
import concourse.bass as bass
import concourse.tile as tile
from concourse import mybir
from concourse.bass_utils import run_bass_kernel_spmd
from concourse.tile import ScopedClock, TileContext


def _patched_drain_and_barrier(self, tick_clock, wait_clock):
    """The walrus build in this container rejects >1 sync wait on one
    instruction ("Too many sync wait commands" on Tile's kernel-tail drain).
    Split the tail-drain waits onto individual SP nops, one wait each."""
    gc = tick_clock.global_clock
    ticks = [int(s) for s in re.findall(r"\d+", repr(gc))]
    for i, t in enumerate(ticks):
        if t > 0:
            nop = self.nc.sync.nop(hint="split_wait", nofuse=True)
            vc = bass_rust.VectorClock()
            vc.require_at_least(i, t)
            wait_clock.add_sem_waits(nop.ins, ScopedClock({None: vc}))
    self.nc.sync.drain()
    self.nc.all_engine_barrier()
    assert self.sems is not None
    popped = self.nc._tile_sem_poison_stack.pop()
    assert popped is self._sem_poison
    self.nc.clear_and_free_semaphores(list(self.sems.allocated().values()))
    self.nc.all_engine_barrier()


TileContext._drain_and_barrier = _patched_drain_and_barrier

_MAX_WAITS = 1  # this walrus build rejects >1 sync wait per instruction


def _split_excess_waits(nc):
    """Move excess sync waits (beyond _MAX_WAITS) from any instruction onto
    freshly inserted same-engine nops placed immediately before it. The
    engine executes the nops (waiting) first, so semantics are unchanged."""
    counter = [0]
    for f in nc.m.functions:
        for blk in f.blocks:
            il = blk.instructions  # live list
            i = 0
            while i < len(il):
                ins = il[i]
                si = ins.sync_info
                if si is not None and len(si.on_wait) > _MAX_WAITS:
                    waits = list(si.on_wait)
                    keep = waits[-_MAX_WAITS:]
                    excess = waits[: -_MAX_WAITS]
                    pos = i
                    for j in range(0, len(excess), _MAX_WAITS):
                        counter[0] += 1
                        nop = mybir.InstNoOp(
                            name=f"I-splitw{counter[0]}", ins=[], outs=[]
                        )
                        nop.engine = ins.engine
                        nop.sync_info = mybir.SyncInfo(
                            on_wait=excess[j : j + _MAX_WAITS], on_update=[]
                        )
                        il.insert(pos, nop)
                        pos += 1
                        i += 1
                    ins.sync_info = mybir.SyncInfo(
                        on_wait=keep, on_update=list(si.on_update)
                    )
                i += 1


N, M, D = 16384, 16384, 32
NCORES = 8
N_LOC = N // NCORES  # 2048 rows per core
BLK = 128  # rows per block (psum partitions)
NBLK = N_LOC // BLK  # 16
CHUNK = 2048  # columns per consumer instruction (4 psum banks)
NCHUNK = M // CHUNK  # 8 per block
MMW = 512  # matmul free width (psum-bank cap: 512 fp32)
SEED_W = 512  # seed max over first SEED_W columns

LN2 = float(np.log(2.0))
A16 = 2.0**7 / LN2  # Schraudolph scale for bf16 bit patterns
C16 = 7.392  # zero-mean calibration (see numcheck2)
B16 = 127 * 2.0**7 - C16

# Per-block chunk pattern: True = ACT (native exp), False = chain (DVE).
# ~62% ACT / 38% chain balances ACT (2.2us/chunk incl accum-readout) vs the
# DVE chain (P1 convert 2.35us + half-pair stt accum ~1.3us per chunk).
_PAT_5A = (True, False, True, False, True, False, True, True)
_BLOCK_PAT = [_PAT_5A for _ in range(NBLK)]
SLOTS = 8  # accum columns reserved per block in s_out (one per chunk)

F32 = mybir.dt.float32
U16 = mybir.dt.uint16
BF16 = mybir.dt.bfloat16
FP8 = mybir.dt.float8e4
Alu = mybir.AluOpType

_cache = {}


def _build_bass():
    nc = bass.Bass()
    xT_d = nc.declare_dram_parameter("xT", [128, 2, N_LOC], FP8, isOutput=False)
    xoT_d = nc.declare_dram_parameter("xoT", [128, 2, M], FP8, isOutput=False)
    negsh_d = nc.declare_dram_parameter("negsh", [BLK, NBLK], F32, isOutput=False)
    bvec_d = nc.declare_dram_parameter("bvec", [BLK, NBLK], F32, isOutput=False)
    s_d = nc.declare_dram_parameter("s_out", [BLK, NBLK * SLOTS], F32, isOutput=True)

    with tile.TileContext(nc) as tc, ExitStack() as ctx:
        singles = ctx.enter_context(tc.tile_pool(name="singles", bufs=1))
        zpool = ctx.enter_context(tc.tile_pool(name="zp", bufs=2))
        psp = ctx.enter_context(tc.tile_pool(name="ps", bufs=2, space="PSUM"))

        xo_sb = singles.tile([128, 2, M], FP8)
        x_sb = singles.tile([128, 2, N_LOC], FP8)
        s_full = singles.tile([BLK, NBLK * SLOTS], F32)
        negsh_full = singles.tile([BLK, NBLK], F32)
        bvec_full = singles.tile([BLK, NBLK], F32)
        junk = singles.tile([128, CHUNK // 2], BF16)

        # Spread input DMAs across engine queues so they land in parallel;
        # the first matmuls depend only on x + vectors + xo piece 0.
        nc.sync.dma_start(out=negsh_full, in_=negsh_d[:, :])
        nc.sync.dma_start(out=bvec_full, in_=bvec_d[:, :])
        nc.sync.dma_start(out=x_sb, in_=xT_d[:, :, :])
        NPIECE = 8
        PW = M // NPIECE
        dma_engines = [nc.sync, nc.gpsimd]
        for p in range(NPIECE):
            dma_engines[p % len(dma_engines)].dma_start(
                out=xo_sb[:, :, p * PW : (p + 1) * PW],
                in_=xoT_d[:, :, p * PW : (p + 1) * PW],
            )

        H = CHUNK // 2
        for b in range(NBLK):
            pat = _BLOCK_PAT[b]
            negsh = negsh_full[:, b : b + 1]
            bvec = bvec_full[:, b : b + 1]
            s_all = s_full[:, b * SLOTS : (b + 1) * SLOTS]
            lhsT = x_sb[:, :, b * BLK : (b + 1) * BLK]
            for ci, is_act in enumerate(pat):
                ps = psp.tile([BLK, CHUNK], F32, tag="ps")
                for c in range(CHUNK // MMW):
                    j0 = ci * CHUNK + c * MMW
                    nc.tensor.matmul(
                        out=ps[:, c * MMW : (c + 1) * MMW],
                        lhsT=lhsT,
                        rhs=xo_sb[:, :, j0 : j0 + MMW],
                        start=True,
                        stop=True,
                        perf_mode=mybir.MatmulPerfMode.DoubleRow,
                    )
                if is_act:
                    nc.scalar.activation(
                        out=ps,
                        in_=ps,
                        func=mybir.ActivationFunctionType.Exp,
                        bias=negsh,
                        scale=1.0,
                        accum_out=s_all[:, ci : ci + 1],
                    )
                else:
                    zt = zpool.tile([128, CHUNK], U16, tag="z")
                    nc.vector.tensor_scalar(
                        out=zt,
                        in0=ps,
                        scalar1=float(A16),
                        scalar2=bvec,
                        op0=Alu.mult,
                        op1=Alu.add,
                    )
                    # half-pair add over the chunk's bf16 bit patterns:
                    # TT-class reads both streams in parallel, so even the
                    # 1x uop costs only CHUNK/2 cycles per chunk.
                    nc.vector.scalar_tensor_tensor(
                        out=junk[:, :H],
                        in0=zt[:, 0:H].bitcast(BF16),
                        scalar=1.0,
                        in1=zt[:, H:CHUNK].bitcast(BF16),
                        op0=Alu.mult,
                        op1=Alu.add,
                        accum_out=s_all[:, ci : ci + 1],
                    )
            nc.sync.dma_start(
                out=s_d[:, b * SLOTS : (b + 1) * SLOTS],
                in_=s_all,
            )

    _split_excess_waits(nc)
    return nc


def _get_nc():
    if "nc" not in _cache:
        _cache["nc"] = _build_bass()
    return _cache["nc"]


def _fp8(v):
    return v.astype(ml_dtypes.float8_e4m3)


def _prep_inputs(x, x_w, x_obs, x_obs_w):
    x = np.ascontiguousarray(x, dtype=np.float32)
    x_obs = np.ascontiguousarray(x_obs, dtype=np.float32)
    x_obs_w = np.ascontiguousarray(x_obs_w, dtype=np.float32)

    c = np.sum(x_obs * x_obs, axis=1, dtype=np.float32)
    a = (-2.0 * c + np.log(x_obs_w)).astype(np.float32)

    # fp8 compensated splits
    x4 = 4.0 * x
    xh = _fp8(x4)
    xl = _fp8(x4 - xh.astype(np.float32))
    xoh = _fp8(x_obs)
    xol = _fp8(x_obs - xoh.astype(np.float32))
    a1 = _fp8(a)
    a2 = _fp8(a - a1.astype(np.float32))
    a3 = _fp8(a - a1.astype(np.float32) - a2.astype(np.float32))
    a4 = _fp8(
        a - a1.astype(np.float32) - a2.astype(np.float32) - a3.astype(np.float32)
    )

    # xoT[p, s, j]: s0 = [hi,lo,hi,lo][p//32](xo)[j, p%32]; s1 p<4 = a_{p+1}
    xoT = np.zeros((128, 2, M), dtype=ml_dtypes.float8_e4m3)
    xoT[0:32, 0, :] = xoh.T
    xoT[32:64, 0, :] = xol.T
    xoT[64:96, 0, :] = xoh.T
    xoT[96:128, 0, :] = xol.T
    xoT[0, 1, :] = a1
    xoT[1, 1, :] = a2
    xoT[2, 1, :] = a3
    xoT[3, 1, :] = a4

    # Host-side LSE shift: exact max of T over the first SEED_W columns.
    # On this data max_j T - shift <= ~70, well under both the fp32 exp
    # range (ACT path) and the uint16 bitcast ceiling (~89.5).
    T_seed = (4.0 * (x @ x_obs[:SEED_W].T) + a[None, :SEED_W]).astype(np.float32)
    shift = T_seed.max(axis=1)  # [N]

    one = np.ones((), dtype=ml_dtypes.float8_e4m3)
    in_maps = []
    for core in range(NCORES):
        sl = slice(core * N_LOC, (core + 1) * N_LOC)
        # xT[p, s, m]: s0 = [hi,hi,lo,lo][p//32](4x)[m, p%32]; s1 p<4 = 1
        xT = np.zeros((128, 2, N_LOC), dtype=ml_dtypes.float8_e4m3)
        xT[0:32, 0, :] = xh[sl].T
        xT[32:64, 0, :] = xh[sl].T
        xT[64:96, 0, :] = xl[sl].T
        xT[96:128, 0, :] = xl[sl].T
        xT[0:4, 1, :] = one
        sh = shift[sl].reshape(NBLK, BLK).T  # [128, NBLK]
        negsh = np.ascontiguousarray(-sh, dtype=np.float32)
        bvec = np.ascontiguousarray(
            (np.float32(B16) - np.float32(A16) * sh), dtype=np.float32
        )
        in_maps.append({"xT": xT, "xoT": xoT, "negsh": negsh, "bvec": bvec})
    return in_maps, shift


def kernel(x, x_w, x_obs, x_obs_w, _trace=False, _tmpdir=None):
    nc = _get_nc()
    in_maps, shift = _prep_inputs(x, x_w, x_obs, x_obs_w)
    res = run_bass_kernel_spmd(
        nc,
        in_maps,
        core_ids=list(range(NCORES)),
        trace=_trace,
        tmpdir=_tmpdir,
    )
    _cache["last_results"] = res
    # host epilogue (fp64): lse_i = shift_i + log(S_i) + b_i
    x = np.ascontiguousarray(x, dtype=np.float32)
    x_w64 = np.ascontiguousarray(x_w, dtype=np.float32).astype(np.float64)
    r = np.sum(x.astype(np.float64) * x, axis=1)
    total = float(np.dot(-2.0 * r, x_w64))
    for core in range(NCORES):
        out = res.results[core]
        s = out["s_out"].astype(np.float64).reshape(BLK, NBLK, SLOTS)
        S = s.sum(axis=2)  # every chunk owns one accum slot
        sl = slice(core * N_LOC, (core + 1) * N_LOC)
        sh = shift[sl].astype(np.float64).reshape(NBLK, BLK).T
        lse = sh + np.log(S)
        w_arr = x_w64[sl].reshape(NBLK, BLK).T
        total += float((lse * w_arr).sum())
    return np.asarray(-total, dtype=np.float32)


# revision 14
# speedup vs baseline: 1.4302x; 1.0164x over previous
"""Trainium2 Bass kernel for nn_ConstrainLoss (weighted logsumexp over a
Gaussian-kernel cost matrix, dotted with row weights -> scalar).

Math:
    sq_ij = |x_i - xo_j|^2          (relu clamp in the reference never fires)
    C_ij  = -2*sq_ij + log(w_obs_j)          (inv_two_s2 == 2.0)
          = 4*x_i.xo_j + a_j + b_i
      a_j = -2*|xo_j|^2 + log(w_obs_j)
      b_i = -2*|x_i|^2            (pulls out of the LSE entirely -> host term)
    out   = -sum_i x_w_i * (b_i + logsumexp_j(T_ij)),  T_ij = 4*x_i.xo_j + a_j

Device kernel (per core, rows sharded 2048/core):
    T tile: fp8e4 DoubleRow matmul (2 cols/cycle). Contraction packed into
      [128 partitions, 2 k-subtiles]: subtile 0 = hi/lo compensated splits
      (hi*hi, hi*lo, lo*hi, lo*lo of 4x and xo, 32 rows each); subtile 1 =
      4 fp8 refinement rows of a_j (x-side ones). |T| error ~0.1-0.3 which
      the final weighted sum averages away (validated 2e-4 rel on host sim).
    Columns of each 128-row block are split between two exp pipelines:
      - ACT chunks: ScalarE exp with bias=-shift and fused accum_out
        (1 elem/cycle/lane) straight from PSUM.
      - chain chunks: DVE tensor_scalar computes y16 = A16*T + (B16-A16*s)
        with output dtype uint16: the fp32->uint16 convert saturates
        negatives to 0, which IS the Schraudolph clamp. The uint16 holds the
        bf16 bit pattern of ~exp(T-s) (A16=2^7/ln2, B16 zero-mean
        calibrated). A second DVE tensor_scalar over the bf16-bitcast view
        (16-bit 2x/4x mode) accumulates the block's chain columns into one
        accum col.
    Splitting exp across ACT+DVE is what beats the ACT-only baseline (ACT
    was 92% busy at 268us); fp8 halves PE time so the stalled-PE mid-pstate
    (1.2GHz) still keeps PE ahead of the consumers.
    lse_i = shift_i + ln(S) on host in fp64 (S = sum of accum cols).

Host: result = -(sum_cores sum(lse*w) + sum_i b_i*x_w_i)
"""

import sys

if "/opt/trn_rl_repo" not in sys.path:
    sys.path.insert(0, "/opt/trn_rl_repo")

import re
from contextlib import ExitStack

import ml_dtypes
import numpy as np

import bass_rust
import concourse.bass as bass
import concourse.tile as tile
from concourse import mybir
from concourse.bass_utils import run_bass_kernel_spmd
from concourse.tile import ScopedClock, TileContext


def _patched_drain_and_barrier(self, tick_clock, wait_clock):
    """The walrus build in this container rejects >1 sync wait on one
    instruction ("Too many sync wait commands" on Tile's kernel-tail drain).
    Split the tail-drain waits onto individual SP nops, one wait each."""
    gc = tick_clock.global_clock
    ticks = [int(s) for s in re.findall(r"\d+", repr(gc))]
    for i, t in enumerate(ticks):
        if t > 0:
            nop = self.nc.sync.nop(hint="split_wait", nofuse=True)
            vc = bass_rust.VectorClock()
            vc.require_at_least(i, t)
            wait_clock.add_sem_waits(nop.ins, ScopedClock({None: vc}))
    self.nc.sync.drain()
    self.nc.all_engine_barrier()
    assert self.sems is not None
    popped = self.nc._tile_sem_poison_stack.pop()
    assert popped is self._sem_poison
    self.nc.clear_and_free_semaphores(list(self.sems.allocated().values()))
    self.nc.all_engine_barrier()


TileContext._drain_and_barrier = _patched_drain_and_barrier

_MAX_WAITS = 1  # this walrus build rejects >1 sync wait per instruction


def _split_excess_waits(nc):
    """Move excess sync waits (beyond _MAX_WAITS) from any instruction onto
    freshly inserted same-engine nops placed immediately before it. The
    engine executes the nops (waiting) first, so semantics are unchanged."""
    counter = [0]
    for f in nc.m.functions:
        for blk in f.blocks:
            il = blk.instructions  # live list
            i = 0
            while i < len(il):
                ins = il[i]
                si = ins.sync_info
                if si is not None and len(si.on_wait) > _MAX_WAITS:
                    waits = list(si.on_wait)
                    keep = waits[-_MAX_WAITS:]
                    excess = waits[: -_MAX_WAITS]
                    pos = i
                    for j in range(0, len(excess), _MAX_WAITS):
                        counter[0] += 1
                        nop = mybir.InstNoOp(
                            name=f"I-splitw{counter[0]}", ins=[], outs=[]
                        )
                        nop.engine = ins.engine
                        nop.sync_info = mybir.SyncInfo(
                            on_wait=excess[j : j + _MAX_WAITS], on_update=[]
                        )
                        il.insert(pos, nop)
                        pos += 1
                        i += 1
                    ins.sync_info = mybir.SyncInfo(
                        on_wait=keep, on_update=list(si.on_update)
                    )
                i += 1


N, M, D = 16384, 16384, 32
NCORES = 8
N_LOC = N // NCORES  # 2048 rows per core
BLK = 128  # rows per block (psum partitions)
NBLK = N_LOC // BLK  # 16
CHUNK = 2048  # columns per consumer instruction (4 psum banks)
NCHUNK = M // CHUNK  # 8 per block
MMW = 512  # matmul free width (psum-bank cap: 512 fp32)
SEED_W = 512  # seed max over first SEED_W columns

LN2 = float(np.log(2.0))
A16 = 2.0**7 / LN2  # Schraudolph scale for bf16 bit patterns
C16 = 7.392  # zero-mean calibration (see numcheck2)
B16 = 127 * 2.0**7 - C16

# Per-block chunk pattern: True = ACT (native exp), False = chain (DVE).
# ~62% ACT / 38% chain balances ACT (2.2us/chunk incl accum-readout) vs the
# DVE chain (P1 convert 2.35us + half-pair stt accum ~1.3us per chunk).
# A-runs stay <=2 everywhere incl. the block boundary (A..A|A..) so neither
# engine starves while the other drains consecutive chunks.
_PAT_5A = (True, False, True, False, True, True, False, True)
_BLOCK_PAT = [_PAT_5A for _ in range(NBLK)]
SLOTS = 8  # accum columns reserved per block in s_out (one per chunk)

F32 = mybir.dt.float32
U16 = mybir.dt.uint16
BF16 = mybir.dt.bfloat16
FP8 = mybir.dt.float8e4
Alu = mybir.AluOpType

_cache = {}


def _build_bass():
    nc = bass.Bass()
    xT_d = nc.declare_dram_parameter("xT", [128, 2, N_LOC], FP8, isOutput=False)
    xoT_d = nc.declare_dram_parameter("xoT", [128, 2, M], FP8, isOutput=False)
    negsh_d = nc.declare_dram_parameter("negsh", [BLK, NBLK], F32, isOutput=False)
    bvec_d = nc.declare_dram_parameter("bvec", [BLK, NBLK], F32, isOutput=False)
    s_d = nc.declare_dram_parameter("s_out", [BLK, NBLK * SLOTS], F32, isOutput=True)

    with tile.TileContext(nc) as tc, ExitStack() as ctx:
        singles = ctx.enter_context(tc.tile_pool(name="singles", bufs=1))
        zpool = ctx.enter_context(tc.tile_pool(name="zp", bufs=2))
        psp = ctx.enter_context(tc.tile_pool(name="ps", bufs=2, space="PSUM"))

        xo_sb = singles.tile([128, 2, M], FP8)
        x_sb = singles.tile([128, 2, N_LOC], FP8)
        s_full = singles.tile([BLK, NBLK * SLOTS], F32)
        negsh_full = singles.tile([BLK, NBLK], F32)
        bvec_full = singles.tile([BLK, NBLK], F32)
        junk = singles.tile([128, CHUNK // 2], BF16)

        # Spread input DMAs across engine queues so they land in parallel.
        # The first chunk needs x block 0 + xoT cols [0, 2048): put those
        # first on separate queues so compute starts ~5us in, not ~20us.
        nc.gpsimd.dma_start(out=negsh_full, in_=negsh_d[:, :])
        nc.gpsimd.dma_start(out=bvec_full, in_=bvec_d[:, :])
        XP = 4  # x pieces (4 blocks each)
        XW = N_LOC // XP
        for p in range(XP):
            nc.gpsimd.dma_start(
                out=x_sb[:, :, p * XW : (p + 1) * XW],
                in_=xT_d[:, :, p * XW : (p + 1) * XW],
            )
        NPIECE = 16
        PW = M // NPIECE
        dma_engines = [nc.sync, nc.scalar]
        for p in range(NPIECE):
            dma_engines[p % len(dma_engines)].dma_start(
                out=xo_sb[:, :, p * PW : (p + 1) * PW],
                in_=xoT_d[:, :, p * PW : (p + 1) * PW],
            )

        H = CHUNK // 2
        for b in range(NBLK):
            pat = _BLOCK_PAT[b]
            negsh = negsh_full[:, b : b + 1]
            bvec = bvec_full[:, b : b + 1]
            s_all = s_full[:, b * SLOTS : (b + 1) * SLOTS]
            lhsT = x_sb[:, :, b * BLK : (b + 1) * BLK]
            for ci, is_act in enumerate(pat):
                ps = psp.tile([BLK, CHUNK], F32, tag="ps")
                for c in range(CHUNK // MMW):
                    j0 = ci * CHUNK + c * MMW
                    nc.tensor.matmul(
                        out=ps[:, c * MMW : (c + 1) * MMW],
                        lhsT=lhsT,
                        rhs=xo_sb[:, :, j0 : j0 + MMW],
                        start=True,
                        stop=True,
                        perf_mode=mybir.MatmulPerfMode.DoubleRow,
                    )
                if is_act:
                    nc.scalar.activation(
                        out=ps,
                        in_=ps,
                        func=mybir.ActivationFunctionType.Exp,
                        bias=negsh,
                        scale=1.0,
                        accum_out=s_all[:, ci : ci + 1],
                    )
                else:
                    zt = zpool.tile([128, CHUNK], U16, tag="z")
                    nc.vector.tensor_scalar(
                        out=zt,
                        in0=ps,
                        scalar1=float(A16),
                        scalar2=bvec,
                        op0=Alu.mult,
                        op1=Alu.add,
                    )
                    # half-pair add over the chunk's bf16 bit patterns:
                    # TT-class reads both streams in parallel, so even the
                    # 1x uop costs only CHUNK/2 cycles per chunk.
                    nc.vector.scalar_tensor_tensor(
                        out=junk[:, :H],
                        in0=zt[:, 0:H].bitcast(BF16),
                        scalar=1.0,
                        in1=zt[:, H:CHUNK].bitcast(BF16),
                        op0=Alu.mult,
                        op1=Alu.add,
                        accum_out=s_all[:, ci : ci + 1],
                    )
            nc.sync.dma_start(
                out=s_d[:, b * SLOTS : (b + 1) * SLOTS],
                in_=s_all,
            )

    _split_excess_waits(nc)
    return nc


def _get_nc():
    if "nc" not in _cache:
        _cache["nc"] = _build_bass()
    return _cache["nc"]


def _fp8(v):
    return v.astype(ml_dtypes.float8_e4m3)


def _prep_inputs(x, x_w, x_obs, x_obs_w):
    x = np.ascontiguousarray(x, dtype=np.float32)
    x_obs = np.ascontiguousarray(x_obs, dtype=np.float32)
    x_obs_w = np.ascontiguousarray(x_obs_w, dtype=np.float32)

    c = np.sum(x_obs * x_obs, axis=1, dtype=np.float32)
    a = (-2.0 * c + np.log(x_obs_w)).astype(np.float32)

    # fp8 compensated splits
    x4 = 4.0 * x
    xh = _fp8(x4)
    xl = _fp8(x4 - xh.astype(np.float32))
    xoh = _fp8(x_obs)
    xol = _fp8(x_obs - xoh.astype(np.float32))
    a1 = _fp8(a)
    a2 = _fp8(a - a1.astype(np.float32))
    a3 = _fp8(a - a1.astype(np.float32) - a2.astype(np.float32))
    a4 = _fp8(
        a - a1.astype(np.float32) - a2.astype(np.float32) - a3.astype(np.float32)
    )

    # xoT[p, s, j]: s0 = [hi,lo,hi,lo][p//32](xo)[j, p%32]; s1 p<4 = a_{p+1}
    xoT = np.zeros((128, 2, M), dtype=ml_dtypes.float8_e4m3)
    xoT[0:32, 0, :] = xoh.T
    xoT[32:64, 0, :] = xol.T
    xoT[64:96, 0, :] = xoh.T
    xoT[96:128, 0, :] = xol.T
    xoT[0, 1, :] = a1
    xoT[1, 1, :] = a2
    xoT[2, 1, :] = a3
    xoT[3, 1, :] = a4

    # Host-side LSE shift: exact max of T over the first SEED_W columns.
    # On this data max_j T - shift <= ~70, well under both the fp32 exp
    # range (ACT path) and the uint16 bitcast ceiling (~89.5).
    T_seed = (4.0 * (x @ x_obs[:SEED_W].T) + a[None, :SEED_W]).astype(np.float32)
    shift = T_seed.max(axis=1)  # [N]

    one = np.ones((), dtype=ml_dtypes.float8_e4m3)
    in_maps = []
    for core in range(NCORES):
        sl = slice(core * N_LOC, (core + 1) * N_LOC)
        # xT[p, s, m]: s0 = [hi,hi,lo,lo][p//32](4x)[m, p%32]; s1 p<4 = 1
        xT = np.zeros((128, 2, N_LOC), dtype=ml_dtypes.float8_e4m3)
        xT[0:32, 0, :] = xh[sl].T
        xT[32:64, 0, :] = xh[sl].T
        xT[64:96, 0, :] = xl[sl].T
        xT[96:128, 0, :] = xl[sl].T
        xT[0:4, 1, :] = one
        sh = shift[sl].reshape(NBLK, BLK).T  # [128, NBLK]
        negsh = np.ascontiguousarray(-sh, dtype=np.float32)
        bvec = np.ascontiguousarray(
            (np.float32(B16) - np.float32(A16) * sh), dtype=np.float32
        )
        in_maps.append({"xT": xT, "xoT": xoT, "negsh": negsh, "bvec": bvec})
    return in_maps, shift


def kernel(x, x_w, x_obs, x_obs_w, _trace=False, _tmpdir=None):
    nc = _get_nc()
    in_maps, shift = _prep_inputs(x, x_w, x_obs, x_obs_w)
    res = run_bass_kernel_spmd(
        nc,
        in_maps,
        core_ids=list(range(NCORES)),
        trace=_trace,
        tmpdir=_tmpdir,
    )
    _cache["last_results"] = res
    # host epilogue (fp64): lse_i = shift_i + log(S_i) + b_i
    x = np.ascontiguousarray(x, dtype=np.float32)
    x_w64 = np.ascontiguousarray(x_w, dtype=np.float32).astype(np.float64)
    r = np.sum(x.astype(np.float64) * x, axis=1)
    total = float(np.dot(-2.0 * r, x_w64))
    for core in range(NCORES):
        out = res.results[core]
        s = out["s_out"].astype(np.float64).reshape(BLK, NBLK, SLOTS)
        S = s.sum(axis=2)  # every chunk owns one accum slot
        sl = slice(core * N_LOC, (core + 1) * N_LOC)
        sh = shift[sl].astype(np.float64).reshape(NBLK, BLK).T
        lse = sh + np.log(S)
        w_arr = x_w64[sl].reshape(NBLK, BLK).T
        total += float((lse * w_arr).sum())
    return np.asarray(-total, dtype=np.float32)
